# revision 36
# baseline (speedup 1.0000x reference)
"""Trainium2 Bass kernel for nn_AZConv2d (fuzzy-rule hyperbolic-geometry message passing).

Self-contained: hardcodes shapes B=8,C=64,H=W=128,R=4,Cout=64; shards batch over 8 cores.

v7 (861us -> 400us vs v2): phase-split pipeline, all-bf16/fp16 matmul streams.
  - Pass A1 (gq only): per row one bf16 stationary [x_hi; x_lo] + 4 small
    matmuls giving gq = (x0+x1+x2)^T(w0+w1+w2) to ~fp32 accuracy (needed:
    theta pairs can be degenerate to ~1e-5); biases folded into the strip
    -granular PSUM->SBUF evac add. No fp32 LDWEIGHTS anywhere.
  - Fields: full-image [128, 4, 130] ops; Sin ops grouped before Exp/Ln ops
    (ACT table swaps); pair-loop temps double-buffered so pairs pipeline.
  - Pass A2 (z): per ROW-PAIR one N=512 matmul ([x0_j; x0_j+1] stationary vs
    block-diag [[wz,0],[0,wz]]); evacs ACT-only so the DVE stays free; z in
    4 strip tiles [128, 256ch, 34rho] fp16 (seam rows duplicated) so stencil
    products depend per-strip, not on the whole-image z; pass A2's pools stay
    open through the stencil (SBUF/PSUM space reuse would serialize phases).
    Its matmuls fill the PE while the DVE does fields; its evacs hide under
    the DVE-bound stencil.
  - Stencil: products on DVE (fp16 2x, ~16us/block is the wall) into
    P[dx][128, 3dy, 4r, 64o, 8rho], rho innermost everywhere; 36-term
    (dx,dy,r) sum as PSUM-accumulating shift matmuls with CONTIGUOUS rhs
    (N=512) and CONTIGUOUS dst acc[128, Cout, BLK] (a transposed matmul dst
    halves the PE rate: 429 vs 216ns measured); ACT evac un-transposes.
  - dedupe_ldweights(): the scheduler emits one LDWEIGHTS per matmul; repeats
    of the identical stationary are rewritten to NoOps post-schedule.
"""
import numpy as np
from contextlib import ExitStack

import concourse.bass as bass
import concourse.tile as tile
from concourse import mybir
from concourse.bass_utils import run_bass_kernel_spmd

F32 = mybir.dt.float32
F16 = mybir.dt.float16
BF16 = mybir.dt.bfloat16
AF = mybir.ActivationFunctionType
OP = mybir.AluOpType

B, C, H, W, R, Cout = 8, 64, 128, 128, 4, 64
L = H * W
NCORE = 8
BLK = 8                 # stencil rows per psum accumulation block
NBLK = H // BLK         # 16
SA = 32                 # phase A strip rows
PI = float(np.pi)
PAIRS = [(0, 1), (1, -1), (1, 0), (1, 1)]   # (dy, dx)
DXI = {-1: 0, 0: 1, 1: 2}

_CACHE = {}


def split_multiwaits(nc):
    """This walrus accepts ONE sync wait per instruction: split extras into
    same-engine NoOps inserted just before the instruction."""
    n = 0
    for bb in nc.main_func.blocks:
        out = []
        for ins in bb.instructions:
            si = ins.sync_info
            if si is not None and len(si.on_wait) > 1:
                waits = list(si.on_wait)
                for w in waits[:-1]:
                    n += 1
                    nop = mybir.InstNoOp(name=f"WSPLIT-{n}")
                    nop.engine = ins.engine
                    nop.sync_info = mybir.SyncInfo(on_wait=[w], on_update=[])
                    out.append(nop)
                ins.sync_info = mybir.SyncInfo(on_wait=[waits[-1]],
                                               on_update=list(si.on_update))
            out.append(ins)
        bb.instructions[:] = out
    return n


def dedupe_ldweights(nc):
    """The tile scheduler emits one LDWEIGHTS per matmul even when many
    consecutive matmuls stream against the identical stationary (e.g. the 12
    shift matmuls per dx group). A reload of the already-loaded array costs
    ~215ns and serializes with the stream. Convert LDWEIGHTS whose weight AP
    (and tile cfg) matches the previous PE weight load into NoOps, keeping
    sync_info so semaphore semantics are unchanged."""
    n = 0
    for bb in nc.main_func.blocks:
        last_sig = None
        out = []
        for ins in bb.instructions:
            tn = type(ins).__name__
            if tn == 'InstLdweights':
                sig = (str(ins.ins[0]),
                       str(getattr(ins, 'tile_position', None)),
                       str(getattr(ins, 'tile_size', None)),
                       str(getattr(ins, 'perf_mode', None)),
                       str(getattr(ins, 'is_transpose', None)))
                if sig == last_sig:
                    n += 1
                    nop = mybir.InstNoOp(name=f"LWDEDUP-{n}")
                    nop.engine = ins.engine
                    nop.sync_info = ins.sync_info
                    out.append(nop)
                    continue
                last_sig = sig
            out.append(ins)
        bb.instructions[:] = out
    return n


def build_program(debug=False):
    nc = bass.Bass()
    xslab_d = nc.dram_tensor("xslab", [128, L], BF16, kind="ExternalInput")
    x2_d = nc.dram_tensor("x2slab", [64, L], BF16, kind="ExternalInput")
    xz_d = nc.dram_tensor("xzslab", [128, L // 2], BF16, kind="ExternalInput")
    wz2_d = nc.dram_tensor("wz2", [128, 512], BF16, kind="ExternalInput")
    wA_d = nc.dram_tensor("wA", [128, 304], BF16, kind="ExternalInput")
    smat_d = nc.dram_tensor("smat", [128, 256], F32, kind="ExternalInput")
    smath_d = nc.dram_tensor("smath", [128, 384], F16, kind="ExternalInput")
    gqb_d = nc.dram_tensor("gqbias", [128, 16], F32, kind="ExternalInput")
    aux_d = nc.dram_tensor("aux", [1, 640], BF16, kind="ExternalInput")
    out_d = nc.dram_tensor("out", [L, Cout], F32, kind="ExternalOutput")
    dbg = None
    if debug:
        dbg = {
            "dbg_gq": nc.dram_tensor("dbg_gq", [128, 16, H + 2], F32,
                                     kind="ExternalOutput")[:],
            "dbg_z": nc.dram_tensor("dbg_z", [128, 256, H + 2], F16,
                                    kind="ExternalOutput")[:],
            "dbg_mu": nc.dram_tensor("dbg_mu", [128, R, H + 2], F32,
                                     kind="ExternalOutput")[:],
            "dbg_wt": nc.dram_tensor("dbg_wt", [128, 3, 3, R, H + 2], F16,
                                     kind="ExternalOutput")[:],
            "dbg_den": nc.dram_tensor("dbg_den", [128, R, H], F32,
                                      kind="ExternalOutput")[:],
        }

    with ExitStack() as ctx:
        tc = ctx.enter_context(tile.TileContext(nc))
        _emit(ctx, tc, xslab_d[:], x2_d[:], xz_d[:], wz2_d[:], wA_d[:],
              smat_d[:], smath_d[:], gqb_d[:], aux_d[:], out_d[:], dbg)
    ndup = dedupe_ldweights(nc)
    split_multiwaits(nc)
    if ndup == 0:
        log_msg = "dedupe_ldweights removed nothing"
    return nc


def _emit(ctx, tc, xslab_d, x2_d, xz_d, wz2_d, wA_d, smat_d, smath_d, gqb_d,
          aux_d, out_d, dbg=None):
    nc = tc.nc

    persist = ctx.enter_context(tc.tile_pool(name="persist", bufs=1))

    # ---------------- persistent tensors ----------------
    wA_sb = persist.tile([128, 304], BF16)
    nc.sync.dma_start(out=wA_sb, in_=wA_d)
    wz2_sb = persist.tile([128, 512], BF16)
    nc.sync.dma_start(out=wz2_sb, in_=wz2_d)
    smat = persist.tile([128, 256], F32)       # [Sp | Sm] f32
    nc.sync.dma_start(out=smat, in_=smat_d)
    smath = persist.tile([128, 384], F16)      # [Sp | Sm | I] fp16
    nc.sync.dma_start(out=smath, in_=smath_d)
    gqbias = persist.tile([128, 16], F32)
    nc.sync.dma_start(out=gqbias, in_=gqb_d)
    aux = persist.tile([1, 640], BF16)         # [ones(128) | pwb_row(512)]
    nc.sync.dma_start(out=aux, in_=aux_d)

    # bias constants for ACT ops ([P,1] APs)
    cb = persist.tile([128, 4], F32)
    nc.vector.memset(cb[:, 0:1], 1e-30)
    nc.vector.memset(cb[:, 1:2], 2e-4)
    nc.vector.memset(cb[:, 2:3], 1e-6)
    nc.vector.memset(cb[:, 3:4], float(np.pi / 2))

    SHIFT = {1: smat[:, 0:128], -1: smat[:, 128:256]}
    SHIFTH = {1: smath[:, 0:128], -1: smath[:, 128:256], 0: smath[:, 256:384]}

    # z in 4 strip tiles [128, 256 ch, 34 rho] fp16 (rho innermost; col c of
    # tile k = image row 32k-1+c, one halo row duplicated at each seam).
    # Strip granularity lets stencil products start as soon as their strip's
    # rows are evacuated instead of waiting for the whole-image z.
    zs = [persist.tile([128, 256, SA + 2], F16, name=f"zs{k}")
          for k in range(H // SA)]
    nc.vector.memset(zs[0][:, :, 0], 0.0)
    nc.vector.memset(zs[H // SA - 1][:, :, SA + 1], 0.0)
    zsv = [t.rearrange("p (r o) c -> p r o c", r=R) for t in zs]
    # gq: [128, 16 fields, 130] f32 (biases pre-added on evac; halo = 0)
    gq = persist.tile([128, 16, H + 2], F32)
    nc.vector.memset(gq[:, :, 0], 0.0)
    nc.vector.memset(gq[:, :, H + 1], 0.0)

    # normalized weights Wt[g, dxi, dyi, r, rho] fp16 + partition-shifted WQ
    Wt = persist.tile([128, 3, 3, R, H + 2], F16)
    WQp = persist.tile([128, 3, R, H + 2], F16)   # dx=+1 group shifted by -1
    WQm = persist.tile([128, 3, R, H + 2], F16)   # dx=-1 group shifted by +1
    WQ = {1: WQp, -1: WQm, 0: Wt[:, 1]}

    # ---------------- phase A2 machinery (z pass): pools open early and stay
    # open through the stencil (space reuse by the stencil P pool would
    # serialize phases). Strip 0 runs INSIDE the gq pass with DVE evacs (the
    # DVE is idle there); strips 1-3 run after the fields with ACT evacs that
    # hide under the DVE-bound stencil. ----------
    phz = ctx.enter_context(tc.tile_pool(name="phZ", bufs=2))
    psZ = ctx.enter_context(tc.tile_pool(name="psZ", bufs=4, space="PSUM"))

    def emit_z_strip(k, on_act):
        q0 = k * SA
        xzw = phz.tile([128, (SA // 2) * 128], BF16, tag="xzw", name="xzw")
        nc.sync.dma_start(
            out=xzw, in_=xz_d[:, (q0 // 2) * 128:(q0 // 2 + SA // 2) * 128])
        for p in range(SA // 2):
            pt2 = psZ.tile([128, 2, 256], F32, tag="pt2", name="pt2")
            nc.tensor.matmul(pt2, xzw[:, p * 128:(p + 1) * 128], wz2_sb,
                             start=True, stop=True, skip_group_check=True)
            srcT = pt2.transpose([0, 2, 1])               # [128, 256, 2]
            dst = zs[k][:, :, 1 + 2 * p:3 + 2 * p]
            if on_act:
                nc.scalar.activation(dst, srcT, AF.Copy)
            else:
                nc.vector.tensor_copy(dst, srcT)
            if p == 0 and k > 0:              # row 32k = prev tile's col 33
                nc.scalar.activation(zs[k - 1][:, :, SA + 1:SA + 2],
                                     srcT[:, :, 0:1], AF.Copy)
            if p == SA // 2 - 1 and k < H // SA - 1:
                # row 32k+31 = next tile's col 0
                nc.scalar.activation(zs[k + 1][:, :, 0:1],
                                     srcT[:, :, 1:2], AF.Copy)

    # ---------------- phase A1: gq (z strip 0 interleaved) ----------------
    with tc.tile_pool(name="phG", bufs=2) as phg, \
         tc.tile_pool(name="psG", bufs=2, space="PSUM") as psG:
        for k in range(H // SA):
            q0 = k * SA
            xw = phg.tile([128, SA * 128], BF16, tag="xw")
            nc.sync.dma_start(out=xw, in_=xslab_d[:, q0 * 128:(q0 + SA) * 128])
            xw2 = phg.tile([64, SA * 128], BF16, tag="xw2")
            nc.sync.dma_start(out=xw2, in_=x2_d[:, q0 * 128:(q0 + SA) * 128])
            gqp = psG.tile([128, SA * 16], F32, tag="gqp")
            for j in range(SA):
                lhsT = xw[:, j * 128:(j + 1) * 128]
                g16 = gqp[:, j * 16:(j + 1) * 16]
                # gq = (x0+x1+x2)^T (w0+w1+w2) to ~fp32 accuracy:
                # [w0;w1]+[w1;w0]+[w2;w2] on [x0;x1], then x2^T w0.
                nc.tensor.matmul(g16, lhsT, wA_sb[:, 256:272],
                                 start=True, stop=False,
                                 skip_group_check=True)
                nc.tensor.matmul(g16, lhsT, wA_sb[:, 272:288],
                                 start=False, stop=False,
                                 skip_group_check=True)
                nc.tensor.matmul(g16, lhsT, wA_sb[:, 288:304],
                                 start=False, stop=False,
                                 skip_group_check=True)
                nc.tensor.matmul(g16, xw2[:, j * 128:(j + 1) * 128],
                                 wA_sb[0:64, 256:272],
                                 start=False, stop=True,
                                 skip_group_check=True)
            # gq strip evac with bias add: psum [32, 16] -> gq [16, 32]
            dstg = gq[:, :, 1 + q0:1 + q0 + SA]
            srcg = gqp.rearrange("p (j c) -> p c j", c=16)
            bcol = gqbias[:, :, None].to_broadcast([128, 16, SA])
            nc.vector.tensor_tensor(dstg, srcg, bcol, op=OP.add)
            if k == 0:
                emit_z_strip(0, on_act=False)

    if dbg is not None:
        nc.sync.dma_start(out=dbg["dbg_gq"], in_=gq)

    # ---------------- fields (full image) ----------------
    with tc.tile_pool(name="phF", bufs=1) as fld, \
         tc.tile_pool(name="psF", bufs=4, space="PSUM") as psF:
        fst = ctx.enter_context(tc.tile_pool(name="fsetup", bufs=1))

        def shift_into(dst_t, src_ap, sgn, dtype_f32, nch):
            """dst[g] = src[g+sgn]; src/dst [128, nch, 130]; 2-rule chunks."""
            step = 2
            for c0 in range(0, nch, step):
                ps = psF.tile([128, step, H + 2], F32, tag="psh")
                if dtype_f32:
                    nc.tensor.matmul(ps, SHIFT[sgn], src_ap[:, c0:c0 + step],
                                     start=True, stop=True,
                                     skip_group_check=True)
                else:
                    nc.tensor.matmul(ps, SHIFTH[sgn], src_ap[:, c0:c0 + step],
                                     start=True, stop=True,
                                     skip_group_check=True)
                nc.scalar.activation(dst_t[:, c0:c0 + step], ps, AF.Copy)

        thw = gq[:, 4:8, :]     # theta + b_th
        # --- theta path first (Sin table) ---
        m1 = fst.tile([128, R, H + 2], F32, tag="m1")
        m2 = fst.tile([128, R, H + 2], F32, tag="m2")
        tred = fst.tile([128, R, H + 2], F32, tag="tred")
        tred2 = fst.tile([128, R, H + 2], F32, tag="tred2")
        s2cF = fld.tile([128, R, H + 2], F32, tag="s2cF")
        c2cF = fld.tile([128, R, H + 2], F32, tag="c2cF")
        nc.vector.tensor_scalar(m1, thw, -PI / 2, None, op0=OP.is_lt)
        nc.vector.tensor_scalar(m2, thw, PI / 2, None, op0=OP.is_gt)
        nc.vector.tensor_tensor(m1, m1, m2, op=OP.subtract)
        nc.vector.scalar_tensor_tensor(out=tred, in0=m1, scalar=PI, in1=thw,
                                       op0=OP.mult, op1=OP.add)
        nc.scalar.activation(s2cF, tred, AF.Sin, scale=2.0)
        nc.vector.tensor_scalar(m1, thw, -0.75 * PI, None, op0=OP.is_lt)
        nc.vector.tensor_scalar(m2, thw, 0.25 * PI, None, op0=OP.is_gt)
        nc.vector.tensor_tensor(m1, m1, m2, op=OP.subtract)
        nc.vector.scalar_tensor_tensor(out=tred2, in0=m1, scalar=PI, in1=thw,
                                       op0=OP.mult, op1=OP.add)
        nc.scalar.activation(c2cF, tred2, AF.Sin, scale=2.0, bias=cb[:, 3:4])

        # --- softmax mu (Exp/Ln table) ---
        eg = fst.tile([128, R, H + 2], F32, tag="eg")
        nc.scalar.activation(eg, gq[:, 0:4, :], AF.Exp)
        nc.vector.memset(eg[:, :, 0], 0.0)
        nc.vector.memset(eg[:, :, H + 1], 0.0)
        zsum = fst.tile([128, H + 2], F32, tag="zsum")
        nc.vector.tensor_tensor(zsum, eg[:, 0], eg[:, 1], op=OP.add)
        nc.vector.tensor_tensor(zsum, zsum, eg[:, 2], op=OP.add)
        nc.vector.tensor_tensor(zsum, zsum, eg[:, 3], op=OP.add)
        rz = fst.tile([128, H + 2], F32, tag="rz")
        nc.scalar.activation(rz, zsum, AF.Ln, bias=cb[:, 0:1])
        nc.scalar.activation(rz, rz, AF.Exp, scale=-1.0)
        mu = fld.tile([128, R, H + 2], F32, tag="mu")
        rzb = rz[:, None, :].to_broadcast([128, R, H + 2])
        nc.vector.tensor_tensor(mu, eg, rzb, op=OP.mult)

        # --- hyper / base fields ---
        uh = fst.tile([128, R, H + 2], F32, tag="uh")
        nc.scalar.activation(uh, gq[:, 12:16, :], AF.Exp)
        ub = fst.tile([128, R, H + 2], F32, tag="ub")
        nc.scalar.activation(ub, gq[:, 8:12, :], AF.Exp)
        Ft = fld.tile([128, R, H + 2], F32, tag="Ft")
        nc.vector.tensor_scalar_add(Ft, uh, 1.0)
        lnf = fst.tile([128, R, H + 2], F32, tag="lnf")
        nc.scalar.activation(lnf, uh, AF.Ln, bias=1.0)
        Gt = fld.tile([128, R, H + 2], F32, tag="Gt")
        nc.scalar.activation(Gt, lnf, AF.Exp, scale=-1.0)
        bt = fld.tile([128, R, H + 2], F32, tag="bt")
        nc.scalar.activation(bt, ub, AF.Ln, bias=1.0)

        if dbg is not None:
            nc.sync.dma_start(out=dbg["dbg_mu"], in_=mu)

        # --- shifted copies ---
        base = {"c2c": c2cF, "s2c": s2cF, "Ft": Ft, "Gt": Gt, "bt": bt}
        shifted = {}
        for name, t in base.items():
            d = {0: t}
            for sgn in (1, -1):
                st = fld.tile([128, R, H + 2], F32, tag=f"{name}s{sgn}")
                shift_into(st, t, sgn, True, R)
                d[sgn] = st
            shifted[name] = d
        mu16 = fld.tile([128, R, H + 2], F16, tag="mu16")
        nc.vector.tensor_copy(mu16, mu)
        mup = fld.tile([128, R, H + 2], F16, tag="mup")
        mum = fld.tile([128, R, H + 2], F16, tag="mum")
        shift_into(mup, mu16, 1, False, R)
        shift_into(mum, mu16, -1, False, R)
        MUSH = {0: mu16, 1: mup, -1: mum}

        ptp = ctx.enter_context(tc.tile_pool(name="ptmp", bufs=2))

        # --- pair loop ---
        comu = [fld.tile([128, R, H + 2], F16, tag=f"comu{i}",
                         name=f"comu{i}") for i in range(4)]
        for cm in comu:
            nc.vector.memset(cm[:, :, 0], 0.0)
            nc.vector.memset(cm[:, :, H + 1], 0.0)
        den = fld.tile([128, R, H], F32, tag="den")
        compat_t = {}
        mirror_t = {}

        def Ctr(t):
            return t[:, :, 1:1 + H]

        for ip, (dy, dx) in enumerate(PAIRS):
            def S(name):
                return shifted[name][dx][:, :, 1 + dy:1 + dy + H]

            # sigma path first: its ACT round-trip (rbp) overlaps the
            # theta chain below instead of stalling the DVE behind it.
            E = ptp.tile([128, R, H], F32, tag="E")
            iE = ptp.tile([128, R, H], F32, tag="iE")
            bp = ptp.tile([128, R, H], F32, tag="bp")
            nc.vector.tensor_tensor(bp, Ctr(bt), S("bt"), op=OP.add)
            nc.vector.tensor_tensor(E, Ctr(Ft), S("Ft"), op=OP.mult)
            nc.vector.tensor_tensor(iE, Ctr(Gt), S("Gt"), op=OP.mult)
            rbp = ptp.tile([128, R, H], F32, tag="rbp")
            nc.scalar.activation(rbp, bp, AF.Ln, bias=cb[:, 1:2])
            nc.scalar.activation(rbp, rbp, AF.Exp, scale=-2.0)
            c2 = ptp.tile([128, R, H], F32, tag="c2")
            s2 = ptp.tile([128, R, H], F32, tag="s2")
            q = ptp.tile([128, R, H], F32, tag="q")
            t1 = ptp.tile([128, R, H], F32, tag="t1")
            nc.vector.tensor_tensor(c2, Ctr(c2cF), S("c2c"), op=OP.add)
            nc.vector.tensor_tensor(s2, Ctr(s2cF), S("s2c"), op=OP.add)
            nc.vector.tensor_tensor(q, c2, c2, op=OP.mult)
            nc.vector.tensor_tensor(t1, s2, s2, op=OP.mult)
            nc.vector.tensor_tensor(q, q, t1, op=OP.add)
            rin = ptp.tile([128, R, H], F32, tag="rin")
            nc.scalar.activation(rin, q, AF.Ln)
            nc.scalar.activation(rin, rin, AF.Exp, scale=-0.5)
            nc.vector.tensor_scalar(rin, rin, 1e6, None, op0=OP.min)
            nc.vector.tensor_tensor(c2, c2, rin, op=OP.mult)
            nc.vector.tensor_tensor(s2, s2, rin, op=OP.mult)
            pu2 = ptp.tile([128, R, H], F32, tag="pu2")
            ps2 = ptp.tile([128, R, H], F32, tag="ps2")
            a1, a2, a3 = dx * dx, dy * dy, dx * dy
            if a3 == 0:
                hc = 0.5 * (a1 - a2)
                nc.vector.tensor_scalar(pu2, c2, hc, 0.5, op0=OP.mult,
                                        op1=OP.add)
                nc.vector.tensor_scalar(ps2, c2, -hc, 0.5, op0=OP.mult,
                                        op1=OP.add)
            else:
                nc.vector.tensor_scalar(pu2, s2, float(a3), 1.0, op0=OP.mult,
                                        op1=OP.add)
                nc.vector.tensor_scalar(ps2, s2, float(-a3), 1.0, op0=OP.mult,
                                        op1=OP.add)
            nc.vector.tensor_tensor(pu2, pu2, iE, op=OP.mult)
            nc.vector.tensor_tensor(ps2, ps2, E, op=OP.mult)
            nc.vector.tensor_tensor(pu2, pu2, ps2, op=OP.add)
            nc.vector.tensor_tensor(pu2, pu2, rbp, op=OP.mult)
            kern = ptp.tile([128, R, H], F32, tag="kern")
            nc.scalar.activation(kern, pu2, AF.Exp, scale=-4.0)

            nc.vector.tensor_tensor(comu[ip][:, :, 1:1 + H], kern, Ctr(mu),
                                    op=OP.mult)
            cp = fld.tile([128, R, H], F32, tag=f"cp{ip}")
            nc.vector.tensor_tensor(
                cp, kern, MUSH[dx][:, :, 1 + dy:1 + dy + H], op=OP.mult)
            compat_t[ip] = cp
            if ip == 0:
                nc.vector.tensor_tensor(den, Ctr(mu), cp, op=OP.add)
            else:
                nc.vector.tensor_tensor(den, den, cp, op=OP.add)
            # mirror compat = comu shifted by (-dy, -dx)
            if dx != 0:
                cst = fld.tile([128, R, H + 2], F16, tag=f"csh{ip}")
                shift_into(cst, comu[ip], -dx, False, R)
                mirror = cst[:, :, 1 - dy:1 - dy + H]
            else:
                mirror = comu[ip][:, :, 1 - dy:1 - dy + H]
            mirror_t[ip] = mirror
            nc.vector.tensor_tensor(den, den, mirror, op=OP.add)

        if dbg is not None:
            nc.sync.dma_start(out=dbg["dbg_den"], in_=den)
        rden = fld.tile([128, R, H], F32, tag="rden")
        nc.scalar.activation(rden, den, AF.Ln, bias=cb[:, 2:3])
        nc.scalar.activation(rden, rden, AF.Exp, scale=-1.0)

        # --- normalized weights into Wt ---
        for ip, (dy, dx) in enumerate(PAIRS):
            nc.vector.tensor_tensor(Wt[:, DXI[dx], 1 + dy, :, 1:1 + H],
                                    compat_t[ip], rden, op=OP.mult)
            nc.vector.tensor_tensor(Wt[:, DXI[-dx], 1 - dy, :, 1:1 + H],
                                    mirror_t[ip], rden, op=OP.mult)
        nc.vector.tensor_tensor(Wt[:, 1, 1, :, 1:1 + H], Ctr(mu), rden,
                                op=OP.mult)
        if dbg is not None:
            nc.sync.dma_start(out=dbg["dbg_wt"], in_=Wt)

        # --- partition-shift dx groups: WQ[dx][g] = Wt[dx-group][g-dx] ---
        for dx, wq in ((1, WQp), (-1, WQm)):
            src = Wt[:, DXI[dx]].rearrange("p a r c -> p (a r) c")
            dst = wq.rearrange("p a r c -> p (a r) c")
            shift_into(dst, src, -dx, False, 3 * R)

    # ---------------- phase A2: z strips 1-3 (strip 0 was emitted before the
    # fields so its ACT evacs precede the field-ACT work and the stencil can
    # start the moment the fields finish) ----------
    for k in range(1, H // SA):
        emit_z_strip(k, on_act=True)

    # ---------------- stencil ----------------
    with tc.tile_pool(name="phC", bufs=2) as pc, \
         tc.tile_pool(name="psC", bufs=2, space="PSUM") as psC:
        for b in range(NBLK):
            r0 = b * BLK
            P = {}
            for dx in (-1, 0, 1):
                Pt = pc.tile([128, 3, R, Cout, BLK], F16, tag=f"P{dx}")
                for dyi, dy in enumerate((-1, 0, 1)):
                    wb = WQ[dx][:, dyi, :, None,
                                1 + r0:1 + r0 + BLK].to_broadcast(
                        [128, R, Cout, BLK])
                    kS = b // (SA // BLK)
                    c0 = r0 + dy - SA * kS + 1
                    nc.vector.tensor_tensor(
                        Pt[:, dyi],
                        zsv[kS][:, :, :, c0:c0 + BLK],
                        wb, op=OP.mult)
                P[dx] = Pt

            # acc memory is [Cout, BLK] so the matmul dst is CONTIGUOUS
            # (a transposed dst AP halves the PE stream rate); the ACT
            # evacuation un-transposes into [BLK, Cout] for the output DMA.
            acc = psC.tile([128, Cout, BLK], F32, tag="acc")
            n = 0
            for dx in (-1, 0, 1):
                for dyi in range(3):
                    for r in range(R):
                        rhs = P[dx][:, dyi, r]       # [128, 64, 8] contiguous
                        nc.tensor.matmul(acc, SHIFTH[dx], rhs,
                                         start=(n == 0), stop=False,
                                         skip_group_check=True)
                        n += 1
            nc.tensor.matmul(acc, aux[:, 0:128], aux[:, 128:640],
                             start=False, stop=True, skip_group_check=True)
            stg = pc.tile([128, BLK, Cout], F32, tag="stg")
            nc.scalar.activation(stg, acc.transpose([0, 2, 1]), AF.Copy)
            dst = out_d[r0 * 128:(r0 + BLK) * 128, :].rearrange(
                "(rho g) o -> g rho o", g=128)
            nc.sync.dma_start(out=dst, in_=stg)


def _host_prep(inputs):
    import ml_dtypes
    bf = ml_dtypes.bfloat16
    x = np.asarray(inputs["x"], np.float32)
    gate_w = np.asarray(inputs["gate_w"], np.float32)
    gate_b = np.asarray(inputs["gate_b"], np.float32)
    value_w = np.asarray(inputs["value_w"], np.float32)
    geom_w = np.asarray(inputs["geom_w"], np.float32)
    geom_b = np.asarray(inputs["geom_b"], np.float32)
    pw_w = np.asarray(inputs["pw_w"], np.float32)
    pw_b = np.asarray(inputs["pw_b"], np.float32)

    M = pw_w.reshape(Cout, R, C).transpose(1, 0, 2) @ value_w      # [R,Cout,C]
    wz = M.transpose(2, 0, 1).reshape(C, R * Cout)                 # [C, 256]
    wgq = np.concatenate([gate_w.T, geom_w.T], axis=1)             # [C, 16]

    xf = np.ascontiguousarray(x.reshape(B, C, L))
    xh = xf.astype(bf)
    xl = (xf - xh.astype(np.float32)).astype(bf)
    x2 = (xf - xh.astype(np.float32) - xl.astype(np.float32)).astype(bf)
    xslab = np.concatenate([xh, xl], axis=1)                       # [B,128,L]

    w2h = wgq.astype(bf)
    w2l = (wgq - w2h.astype(np.float32)).astype(bf)
    w2q = (wgq - w2h.astype(np.float32) - w2l.astype(np.float32)).astype(bf)
    wA = np.zeros((128, 304), np.float32)
    wA[0:64, 0:256] = wz
    wA[0:64, 256:272] = w2h.astype(np.float32)
    wA[64:128, 256:272] = w2l.astype(np.float32)
    wA[0:64, 272:288] = w2l.astype(np.float32)
    wA[64:128, 272:288] = w2h.astype(np.float32)
    wA[0:64, 288:304] = w2q.astype(np.float32)
    wA[64:128, 288:304] = w2q.astype(np.float32)

    smat = np.zeros((128, 384), np.float32)
    for g in range(128):   # Sp[k, g] = 1 iff k = g+1 ; Sm[k, g] = 1 iff k=g-1
        if g + 1 < 128:
            smat[g + 1, g] = 1.0
        if g - 1 >= 0:
            smat[g - 1, 128 + g] = 1.0
        smat[g, 256 + g] = 1.0

    aux = np.zeros((1, 640), np.float32)
    aux[0, 0:128] = 1.0
    aux[0, 128:640] = np.repeat(pw_b, BLK)   # acc is [Cout, BLK] o-major

    # z-pass: stationary holds [x_hi row 2p ; x_hi row 2p+1]
    xhr = xh.astype(np.float32).reshape(B, C, H, W)
    xz = np.concatenate([xhr[:, :, 0::2, :], xhr[:, :, 1::2, :]],
                        axis=1).reshape(B, 128, L // 2).astype(bf)
    wz2 = np.zeros((128, 512), np.float32)
    wz2[0:64, 0:256] = wz
    wz2[64:128, 256:512] = wz

    gqb_cols = np.concatenate([gate_b, geom_b])                    # [16]
    gqbias = np.tile(gqb_cols[None, :], (128, 1)).astype(np.float32)

    return {
        "xslab": xslab,
        "x2slab": x2,
        "xzslab": xz,
        "wz2": wz2.astype(bf),
        "wA": wA.astype(bf),
        "smat": smat[:, 0:256].copy(),
        "smath": smat.astype(np.float16),
        "gqbias": gqbias,
        "aux": aux.astype(bf),
    }


def make_in_maps(inputs):
    h = _host_prep(inputs)
    return [{"xslab": h["xslab"][b], "x2slab": h["x2slab"][b],
             "xzslab": h["xzslab"][b], "wz2": h["wz2"], "wA": h["wA"],
             "smat": h["smat"], "smath": h["smath"], "gqbias": h["gqbias"],
             "aux": h["aux"]} for b in range(B)]


def kernel(**inputs) -> np.ndarray:
    if "nc" not in _CACHE:
        _CACHE["nc"] = build_program()
    nc = _CACHE["nc"]
    in_maps = make_in_maps(inputs)
    res = run_bass_kernel_spmd(nc, in_maps, core_ids=list(range(NCORE)))
    out = np.stack([
        res.results[b]["out"].reshape(H, W, Cout).transpose(2, 0, 1)
        for b in range(B)
    ])
    return out.astype(np.float32)
